# revision 31
# baseline (speedup 1.0000x reference)
"""Trainium2 Bass kernel for windowed (block-diagonal) multi-head attention.

Problem nn_Attention_17059610099953:
  x: (8, 1936, 384) tokens of a (B=2, t=4, H=44, W=44) volume; 10x10 spatial
  windows (padded to 50x50 -> 5x5 grid), each window = t*10*10 = 400 tokens of
  12-head attention (head_dim 32), followed by an output projection.

Sharding: 50 windows = 32 full (400 real tokens) + 16 edge (160) + 2 corner
(64). Each of the 8 NeuronCores processes 4 full windows + up to 3
edge/corner windows (compacted + padded to 160 tokens; the softmax
denominator is corrected by +240 to match the reference's 400-slot windows,
whose zero padding tokens each contribute exp(0)=1).

v2 pipeline (~243-270us vs the v1 baseline at ~346us):
  - All matmul operands are bf16 (x, qkv/proj weights, q, k, v, softmax
    weights, attention output). This is the single biggest win: bf16 halves
    the SBUF stream bytes per PE column and enables fast weight loads (FWL),
    where fp32r ran MMs at ~500-620ns and weight loads at ~2.5x bf16 cost.
    PSUM accumulation stays fp32; the final output is stored fp32.
  - The PE instruction stream is kept dense: the QKV projections of window
    w+1 and the normalize/project tail of window w-1 are interleaved between
    the per-head QK/exp/PV steps of window w, so the Tensor engine rarely
    starves while the Scalar engine drains softmax exps (PE idle gaps >~1us
    drop the HAM clock gate to K=4/8 = 1.2 GHz for tens of us).
  - PV is column-tiled: heads 2p/2p+1 run concurrently in array column
    strips (tile_position (0,0)/(0,64)), halving PV wall time. The 33rd
    stationary column is the all-ones softmax-denominator row, which rides
    the PV pass for free.
  - exp for 2 of 12 heads is a Schraudolph bit-trick on the Vector engine
    (es.bitcast(int16) = int16(a*scale*x + b)) to unload the Scalar engine;
    scores span only [-1.6, 1.6] and total output error stays ~1.0e-2 of
    output scale (tolerance 2e-2). GpSimd cannot read PSUM, so it only runs
    the SBUF-side rescale muls and gather/broadcast DMAs.
  - Normalization uses reciprocal_approx_fast and splits the 12 per-head
    rescale muls between Vector and GpSimd.
"""
import os
import sys

for _p in ("/opt/trn_rl_repo",):
    if os.path.isdir(_p) and _p not in sys.path:
        sys.path.append(_p)

import numpy as np
import ml_dtypes

import concourse.bass as bass
import concourse.bacc as bacc
import concourse.mybir as mybir
import concourse.tile as tile

F32 = mybir.dt.float32
F32R = mybir.dt.float32r
BF16 = mybir.dt.bfloat16
I16 = mybir.dt.int16
AF = mybir.ActivationFunctionType
ALU = mybir.AluOpType

C = 384
NH = 12
HD = 32
SCALE = HD ** -0.5
NF = 4      # full windows per core (n=400)
NS = 3      # small windows per core (padded to n=160)
NFull = 400
NSmall = 160
VW = 33     # V columns per head (32 dims + ones column for the denominator)

# Schraudolph exp constants, bf16 flavor:
# exp(x) ~= bitcast_bf16(int16(x*EXPA16 + EXPB16))
EXPA16 = 184.6649652337873     # 2^7 / ln 2
EXPB16 = 127.0 * 128.0 - 5.585


def ceil_div(a, b):
    return (a + b - 1) // b


def build_kernel(dve_exp_heads=(5, 11), pool_exp_heads=(), pv_col_tile=True):
    nc = bacc.Bacc("TRN2", target_bir_lowering=False, debug=False, num_devices=8)

    xf = nc.declare_dram_parameter("xf", [NF, 128, 3, NFull], BF16, isOutput=False)
    xs = nc.declare_dram_parameter("xs", [NS, 128, 3, NSmall], BF16, isOutput=False)
    wq = nc.declare_dram_parameter("wq", [128, 3, C], BF16, isOutput=False)
    wk = nc.declare_dram_parameter("wk", [128, 3, C], BF16, isOutput=False)
    wv = nc.declare_dram_parameter("wv", [128, 3, C], BF16, isOutput=False)
    wp = nc.declare_dram_parameter("wp", [128, 3, C], BF16, isOutput=False)
    pb = nc.declare_dram_parameter("pb", [128, 3], F32, isOutput=False)
    zf = nc.declare_dram_parameter("zf", [NF, 128, 3, NFull], F32, isOutput=True)
    zs = nc.declare_dram_parameter("zs", [NS, 128, 3, NSmall], F32, isOutput=True)

    slots = [(s, NFull, xf, zf, s) for s in range(NF)] + \
            [(NF + s, NSmall, xs, zs, s) for s in range(NS)]
    NW = len(slots)

    with tile.TileContext(nc) as tc:
        with tc.tile_pool(name="weights", bufs=1) as wpool, \
             tc.tile_pool(name="xio", bufs=3) as xpool, \
             tc.tile_pool(name="qk", bufs=2) as qkpool, \
             tc.tile_pool(name="vaug", bufs=2) as vpool, \
             tc.tile_pool(name="es", bufs=4) as espool, \
             tc.tile_pool(name="oun", bufs=2) as ounpool, \
             tc.tile_pool(name="oz", bufs=2) as ozpool, \
             tc.tile_pool(name="nrm", bufs=2) as nrmpool, \
             tc.tile_pool(name="nrmbig", bufs=2) as nbpool, \
             tc.tile_pool(name="dscratch", bufs=2, space="DRAM") as dpool, \
             tc.tile_pool(name="ps_s", bufs=2, space="PSUM") as ps_s, \
             tc.tile_pool(name="ps_pv", bufs=2, space="PSUM") as ps_pv, \
             tc.tile_pool(name="ps_mm", bufs=2, space="PSUM") as ps_mm:

            twq = wpool.tile([128, 3, C], BF16, tag="wq")
            twk = wpool.tile([128, 3, C], BF16, tag="wk")
            twv = wpool.tile([128, 3, C], BF16, tag="wv")
            twp = wpool.tile([128, 3, C], BF16, tag="wp")
            tpb = wpool.tile([128, 3], F32, tag="pb")

            class Window:
                def __init__(self, w):
                    self.w = w
                    self.slot, self.n, self.xin, self.zout, self.si = slots[w]
                    self.n_mt = ceil_div(self.n, 128)
                    self.m_sizes = [min(128, self.n - 128 * j)
                                    for j in range(self.n_mt)]
                    self.full = self.n == NFull
                    self.es = {}

                # ---- stage 1: x load + QKV projections (run during w-1) ----
                def load_x(self):
                    self.xt = xpool.tile([128, 3, NFull], BF16, tag="xt",
                                         name=f"xt{self.w}")
                    nc.sync.dma_start(out=self.xt[:, :, 0:self.n],
                                      in_=self.xin[self.si])

                def qkv_chunks(self):
                    n = self.n
                    out = []

                    def qk_proj(dst_key, i, self=self):
                        if dst_key not in ("qt", "kt"):
                            raise ValueError
                        if not hasattr(self, dst_key):
                            setattr(self, dst_key,
                                    qkpool.tile([128, 3, NFull], BF16,
                                                tag=dst_key,
                                                name=f"{dst_key}{self.w}"))
                        dst = getattr(self, dst_key)
                        w_t = twq if dst_key == "qt" else twk
                        pmm = ps_mm.tile([128, 512], F32, tag="mm")
                        for kk in range(3):
                            nc.tensor.matmul(pmm[:, 0:n],
                                             w_t[:, kk, 128 * i:128 * i + 128],
                                             self.xt[:, kk, 0:n],
                                             start=(kk == 0), stop=(kk == 2))
                        nc.vector.tensor_copy(dst[:, i, 0:n], pmm[:, 0:n])

                    def v_tile(j, self=self):
                        if not hasattr(self, "vg"):
                            self.vg = vpool.tile([128, 4, NH * VW], BF16,
                                                 tag="vg", name=f"vg{self.w}")
                        mj = self.m_sizes[j]
                        pmm = ps_mm.tile([128, 512], F32, tag="mm")
                        for kk in range(3):
                            nc.tensor.matmul(pmm[0:mj, 0:C],
                                             self.xt[:, kk, 128 * j:128 * j + mj],
                                             twv[:, kk, :],
                                             start=(kk == 0), stop=(kk == 2))
                        vslice = self.vg[0:mj, j, :].rearrange(
                            "p (h c) -> p h c", h=NH)
                        nc.vector.tensor_copy(
                            vslice[:, :, 0:32],
                            pmm[0:mj, 0:C].rearrange("p (h c) -> p h c", h=NH))
                        nc.vector.memset(vslice[:, :, 32:33], 1.0)

                    for dst in ("qt", "kt"):
                        for i in range(3):
                            out.append(lambda d=dst, i=i: qk_proj(d, i))
                    for j in range(self.n_mt):
                        out.append(lambda j=j: v_tile(j))
                    return out

                # ---- stage 2: per-head QK^T + exp ----
                def head(self, h):
                    n, n_mt = self.n, self.n_mt
                    ti, to = h // 4, 32 * (h % 4)
                    es = espool.tile([128, 4, NFull], BF16, tag="es",
                                     name=f"es{self.w}_{h}")
                    self.es[h] = es
                    exp_eng = None
                    if self.full:
                        if h in dve_exp_heads:
                            exp_eng = nc.vector
                        elif h in pool_exp_heads:
                            exp_eng = nc.gpsimd
                    for half in range(ceil_div(n_mt, 2)):
                        pss = ps_s.tile([128, 2, 512], F32, tag="s")
                        jj = [j for j in (2 * half, 2 * half + 1) if j < n_mt]
                        for idx, j in enumerate(jj):
                            mj = self.m_sizes[j]
                            nc.tensor.matmul(
                                pss[0:mj, idx, 0:n],
                                self.kt[to:to + 32, ti, 128 * j:128 * j + mj],
                                self.qt[to:to + 32, ti, 0:n],
                                start=True, stop=True, tile_position=(to, 0))
                        dst = es[:, 2 * half:2 * half + len(jj), 0:n]
                        src = pss[:, 0:len(jj), 0:n]
                        if exp_eng is None:
                            nc.scalar.activation(dst, src, AF.Exp, scale=SCALE)
                        else:
                            exp_eng.tensor_scalar(
                                dst.bitcast(I16), src,
                                SCALE * EXPA16, EXPB16, ALU.mult, ALU.add)

                # ---- stage 3: PV for a head pair, column-tiled ----
                def pv_pair(self, p):
                    n, n_mt = self.n, self.n_mt
                    h0, h1 = 2 * p, 2 * p + 1
                    if not hasattr(self, "oun"):
                        self.oun = ounpool.tile([128, 6, NFull], F32R,
                                                tag="oun", name=f"oun{self.w}")
                    es0, es1 = self.es.pop(h0), self.es.pop(h1)
                    if pv_col_tile:
                        ppv = ps_pv.tile([128, 512], F32, tag="pv")
                        for j in range(n_mt):
                            mj = self.m_sizes[j]
                            nc.tensor.matmul(
                                ppv[0:33, 0:n],
                                self.vg[0:mj, j, VW * h0:VW * h0 + VW],
                                es0[0:mj, j, 0:n],
                                start=(j == 0), stop=(j == n_mt - 1),
                                tile_position=(0, 0), skip_group_check=True)
                            nc.tensor.matmul(
                                ppv[64:97, 0:n],
                                self.vg[0:mj, j, VW * h1:VW * h1 + VW],
                                es1[0:mj, j, 0:n],
                                start=(j == 0), stop=(j == n_mt - 1),
                                tile_position=(0, 64), skip_group_check=True)
                        nc.vector.tensor_copy(self.oun[0:33, p, 0:n],
                                              ppv[0:33, 0:n])
                        nc.vector.tensor_copy(self.oun[64:97, p, 0:n],
                                              ppv[64:97, 0:n])
                    else:
                        for a, (hh, ee) in enumerate(((h0, es0), (h1, es1))):
                            ppv = ps_pv.tile([128, 512], F32, tag="pv")
                            for j in range(n_mt):
                                mj = self.m_sizes[j]
                                nc.tensor.matmul(
                                    ppv[0:33, 0:n],
                                    self.vg[0:mj, j, VW * hh:VW * hh + VW],
                                    ee[0:mj, j, 0:n],
                                    start=(j == 0), stop=(j == n_mt - 1))
                            nc.vector.tensor_copy(
                                self.oun[64 * a:64 * a + 33, p, 0:n],
                                ppv[0:33, 0:n])

                # ---- stage 4: normalize + project + store (run during w+1) --
                def t_dal(self):
                    n = self.n
                    self.dal = nrmpool.tile([12, NFull], F32, tag="dal",
                                            name=f"dal{self.w}")
                    nc.sync.dma_start(out=self.dal[0:6, 0:n],
                                        in_=self.oun[32:33, :, 0:n].bitcast(F32))
                    nc.sync.dma_start(out=self.dal[6:12, 0:n],
                                        in_=self.oun[96:97, :, 0:n].bitcast(F32))

                def t_rcp(self):
                    n = self.n
                    rcp = nrmpool.tile([12, NFull], F32, tag="rcp",
                                       name=f"rcp{self.w}")
                    self.rcp = rcp
                    src = self.dal
                    if not self.full:
                        dfl = nrmpool.tile([12, NFull], F32, tag="dfl",
                                           name=f"dfl{self.w}")
                        nc.vector.tensor_scalar_add(
                            dfl[:, 0:n], self.dal[:, 0:n],
                            float(NFull - NSmall))
                        src = dfl
                    nc.vector.reciprocal_approx_fast(rcp[:, 0:n], src[:, 0:n])

                def _bca_tile(self):
                    if not hasattr(self, "bca"):
                        # bca[64a+b, p, :] = 1/den of head 2p+a, so the mul
                        # input bases match oun's (same-start-partition rule)
                        self.bca = nbpool.tile([128, 6, NFull], F32, tag="bca",
                                               name=f"bca{self.w}")

                def t_bcast(self):
                    n = self.n
                    dsc = dpool.tile([12, NFull], F32, tag="dsc",
                                     name=f"dsc{self.w}")
                    nc.sync.dma_start(out=dsc[:, 0:n], in_=self.rcp[:, 0:n])
                    self._bca_tile()
                    for a in range(2):
                        nc.sync.dma_start(
                            out=self.bca[64 * a:64 * a + 32, :, 0:n],
                            in_=dsc[None, 6 * a:6 * a + 6, 0:n]
                            .to_broadcast((32, 6, n)))

                def tail_half(self, half):
                    # half-batched dal->rcp->broadcast chain for pairs
                    # 3*half..3*half+2, used for the last window so its tail
                    # pipelines into the head loop instead of serializing
                    # after it (5 DMA triggers per half)
                    n = self.n
                    p0 = 3 * half
                    dal_h = nrmpool.tile([6, NFull], F32, tag="dalh",
                                         name=f"dalh{self.w}_{half}")
                    nc.sync.dma_start(out=dal_h[0:3, 0:n],
                                        in_=self.oun[32:33, p0:p0 + 3, 0:n]
                                        .bitcast(F32))
                    nc.sync.dma_start(out=dal_h[3:6, 0:n],
                                        in_=self.oun[96:97, p0:p0 + 3, 0:n]
                                        .bitcast(F32))
                    src = dal_h
                    if not self.full:
                        dfl_h = nrmpool.tile([6, NFull], F32, tag="dflh",
                                             name=f"dflh{self.w}_{half}")
                        nc.vector.tensor_scalar_add(
                            dfl_h[:, 0:n], dal_h[:, 0:n],
                            float(NFull - NSmall))
                        src = dfl_h
                    rcp_h = nrmpool.tile([6, NFull], F32, tag="rcph",
                                         name=f"rcph{self.w}_{half}")
                    nc.vector.reciprocal_approx_fast(rcp_h[:, 0:n],
                                                     src[:, 0:n])
                    dsc_h = dpool.tile([6, NFull], F32, tag="dsch",
                                       name=f"dsch{self.w}_{half}")
                    nc.sync.dma_start(out=dsc_h[:, 0:n], in_=rcp_h[:, 0:n])
                    self._bca_tile()
                    for a in range(2):
                        nc.sync.dma_start(
                            out=self.bca[64 * a:64 * a + 32, p0:p0 + 3, 0:n],
                            in_=dsc_h[None, 3 * a:3 * a + 3, 0:n]
                            .to_broadcast((32, 3, n)))

                def t_mul(self, h):
                    n = self.n
                    ti, to = h // 4, 32 * (h % 4)
                    p, a = h // 2, h % 2
                    if not hasattr(self, "ot"):
                        self.ot = ozpool.tile([128, 3, NFull], BF16,
                                              tag="ot", name=f"ot{self.w}")
                    eng = nc.vector if h % 2 == 0 else nc.gpsimd
                    eng.tensor_mul(
                        self.ot[to:to + 32, ti, 0:n],
                        self.oun[64 * a:64 * a + 32, p, 0:n],
                        self.bca[64 * a:64 * a + 32, p, 0:n].bitcast(F32R))

                def t_proj(self, i):
                    n = self.n
                    if not hasattr(self, "zt"):
                        self.zt = ozpool.tile([128, 3, NFull], F32,
                                              tag="zt", name=f"zt{self.w}")
                    pmm = ps_mm.tile([128, 512], F32, tag="mm")
                    for kk in range(3):
                        nc.tensor.matmul(pmm[:, 0:n],
                                         twp[:, kk, 128 * i:128 * i + 128],
                                         self.ot[:, kk, 0:n],
                                         start=(kk == 0), stop=(kk == 2))
                    nc.vector.tensor_scalar_add(self.zt[:, i, 0:n],
                                                pmm[:, 0:n],
                                                tpb[:, i:i + 1])

                def t_store(self):
                    nc.sync.dma_start(out=self.zout[self.si],
                                      in_=self.zt[:, :, 0:self.n])

                def tail_chunks(self):
                    out = [self.t_dal, self.t_rcp, self.t_bcast]
                    for h in range(NH):
                        out.append(lambda h=h: self.t_mul(h))
                    for i in range(3):
                        out.append(lambda i=i: self.t_proj(i))
                    out.append(self.t_store)
                    return out

            wins = [Window(w) for w in range(NW)]

            # ---- prologue: x0 + first weights on the sync queue, the
            # rest on the gpsimd queue so transfers overlap ----
            wins[0].load_x()
            for t, src in ((twq, wq), (twk, wk), (twv, wv)):
                nc.sync.dma_start(out=t[:], in_=src[:])
            for t, src in ((twp, wp), (tpb, pb)):
                nc.gpsimd.dma_start(out=t[:], in_=src[:])
            for c in wins[0].qkv_chunks():
                c()

            for w in range(NW):
                cur = wins[w]
                filler = []
                reserved = []
                if w + 1 < NW:
                    nxt = wins[w + 1]
                    filler.append(nxt.load_x)
                    filler.extend(nxt.qkv_chunks())
                if w > 0:
                    filler.extend(wins[w - 1].tail_chunks())
                # Distribute filler chunks across the 12 head slots. The
                # chains (x-load before qkv; dal->rcp->bcast->muls->proj)
                # are kept in order; we round-robin merge the two lists so
                # both PE filler (qkv/proj matmuls) and the normalize tail
                # spread evenly across the window.
                nslots = NH
                per_slot = [[] for _ in range(nslots)]
                for idx, c in enumerate(filler):
                    per_slot[min(nslots - 1, idx * nslots // max(1, len(filler)))].append(c)
                last = w == NW - 1
                for h in range(NH):
                    cur.head(h)
                    if h >= 2 and h % 2 == 0:
                        p = (h - 2) // 2
                        cur.pv_pair(p)
                        if last and p == 2:
                            cur.tail_half(0)
                            for hh in range(6):
                                cur.t_mul(hh)
                    for c in per_slot[h]:
                        c()
                cur.pv_pair(5)
                if last:
                    cur.tail_half(1)
                    for hh in range(6, NH):
                        cur.t_mul(hh)
                    for i in range(3):
                        cur.t_proj(i)
                    cur.t_store()

    nc.compile()
    return nc




WS = 10
NH = 12
C = 384
B, T, H, W = 2, 4, 44, 44
HG = WG = 5
NFull = 400
NSmall = 160
NF, NS = 4, 3


def window_partition(x):
    """x: (B*T, H*W, C) -> windows (B, 25, 400, C) padded, plus metadata."""
    ax = x.reshape(B, T, H, W, C)
    pad = WS * HG
    axp = np.zeros((B, T, pad, pad, C), dtype=x.dtype)
    axp[:, :, :H, :W, :] = ax
    axp = axp.reshape(B, T, HG, WS, WG, WS, C)
    axp = axp.transpose(0, 2, 4, 1, 3, 5, 6).reshape(B, HG * WG, T * WS * WS, C)
    return axp


def classify_windows():
    """Return (full_list, small_list) of (b, w, n_valid)."""
    full, small = [], []
    for b in range(B):
        for i in range(HG):
            for j in range(WG):
                w = i * WG + j
                vi = min(WS, H - i * WS)
                vj = min(WS, W - j * WS)
                nv = T * vi * vj
                if vi == WS and vj == WS:
                    full.append((b, w))
                else:
                    small.append((b, w, nv))
    return full, small


def window_token_index(w):
    """For window w, indices of its 400 token slots ordered by (t, wi, wj),
    and validity mask."""
    i, j = w // WG, w % WG
    idx = np.zeros((T, WS, WS), dtype=np.int64)
    valid = np.zeros((T, WS, WS), dtype=bool)
    for t in range(T):
        for a in range(WS):
            for bb in range(WS):
                hh, ww = i * WS + a, j * WS + bb
                ok = (hh < H) and (ww < W)
                valid[t, a, bb] = ok
                idx[t, a, bb] = (t * H + min(hh, H - 1)) * W + min(ww, W - 1)
    return idx.reshape(-1), valid.reshape(-1)


def compact_window_tokens(xw, w):
    """xw: (400, C) padded window tokens (zeros at invalid). Returns
    (n_valid tokens compacted, order) where order lists the valid slot ids."""
    _, valid = window_token_index(w)
    order = np.nonzero(valid)[0]
    return xw[order], order


def shard_inputs(x, qkv_w, proj_w, proj_b):
    """Build per-core in_maps. Returns (in_maps, meta) where meta is used by
    unshard."""
    x = np.asarray(x, dtype=np.float32)
    xw = window_partition(x)           # (B, 25, 400, C)
    full, small = classify_windows()
    assert len(full) == 32 and len(small) == 18

    # per-core assignment: 4 full, and up to 3 small (pad with zero windows)
    full_assign = [full[4 * c:4 * c + 4] for c in range(8)]
    small_assign = [[] for _ in range(8)]
    for k, s in enumerate(small):
        small_assign[k % 8].append(s)
    meta = {"full": full_assign, "small": small_assign, "orders": {}}

    wqT = qkv_w[0:C, :].T.astype(np.float32)      # (C, C): [c, qf]
    wkT = qkv_w[C:2 * C, :].T.astype(np.float32)
    wvT = qkv_w[2 * C:3 * C, :].T.astype(np.float32)
    wpT = proj_w.T.astype(np.float32)

    def wtile(wt):  # (C=384 rows c, C cols f) -> [128, 3, 384]
        return np.ascontiguousarray(
            wt.reshape(3, 128, C).transpose(1, 0, 2)).astype(ml_dtypes.bfloat16)

    in_maps = []
    for c in range(8):
        xf = np.zeros((NF, 128, 3, NFull), dtype=ml_dtypes.bfloat16)
        for s, (b, w) in enumerate(full_assign[c]):
            xt = xw[b, w].T                      # (C, 400)
            xf[s] = xt.reshape(3, 128, NFull).transpose(1, 0, 2).astype(ml_dtypes.bfloat16)
        xs = np.zeros((NS, 128, 3, NSmall), dtype=ml_dtypes.bfloat16)
        for s, (b, w, nv) in enumerate(small_assign[c]):
            toks, order = compact_window_tokens(xw[b, w], w)
            meta["orders"][(b, w)] = order
            xt = np.zeros((C, NSmall), dtype=np.float32)
            xt[:, 0:nv] = toks.T
            xs[s] = xt.reshape(3, 128, NSmall).transpose(1, 0, 2).astype(ml_dtypes.bfloat16)
        in_maps.append({
            "xf": xf, "xs": xs,
            "wq": wtile(wqT), "wk": wtile(wkT), "wv": wtile(wvT),
            "wp": wtile(wpT),
            "pb": np.ascontiguousarray(proj_b.astype(np.float32).reshape(3, 128).T),
        })
    return in_maps, meta


def unshard_outputs(results, meta):
    """results: list of 8 dicts with zf (NF,128,3,400), zs. Return (B*T, H*W, C)."""
    zwin = np.zeros((B, HG * WG, T * WS * WS, C), dtype=np.float32)
    for c in range(8):
        zfc, zsc = results[c]["zf"], results[c]["zs"]
        for s, (b, w) in enumerate(meta["full"][c]):
            zt = zfc[s].transpose(1, 0, 2).reshape(C, NFull)   # (C, 400)
            zwin[b, w] = zt.T
        for s, (b, w, nv) in enumerate(meta["small"][c]):
            zt = zsc[s].transpose(1, 0, 2).reshape(C, NSmall)
            order = meta["orders"][(b, w)]
            zwin[b, w][order] = zt.T[0:nv]
    # reverse window partition
    z = zwin.reshape(B, HG, WG, T, WS, WS, C)
    z = z.transpose(0, 3, 1, 4, 2, 5, 6).reshape(B, T, HG * WS, WG * WS, C)
    z = z[:, :, :H, :W, :]
    return z.reshape(B * T, H * W, C)


_CACHE = {}


def _get_nc():
    if "nc" not in _CACHE:
        _CACHE["nc"] = build_kernel()
    return _CACHE["nc"]


def kernel(x, qkv_w, proj_w, proj_b, t=4, H=44, W=44, **_unused):
    from concourse.bass_utils import run_bass_kernel_spmd

    x = np.asarray(x, dtype=np.float32)
    qkv_w = np.asarray(qkv_w, dtype=np.float32)
    proj_w = np.asarray(proj_w, dtype=np.float32)
    proj_b = np.asarray(proj_b, dtype=np.float32)
    in_maps, meta = shard_inputs(x, qkv_w, proj_w, proj_b)
    nc = _get_nc()
    res = run_bass_kernel_spmd(nc, in_maps, list(range(8)))
    return unshard_outputs(res.results, meta)


# revision 33
# speedup vs baseline: 1.1328x; 1.1328x over previous
"""Trainium2 Bass kernel for windowed (block-diagonal) multi-head attention.

Problem nn_Attention_17059610099953:
  x: (8, 1936, 384) tokens of a (B=2, t=4, H=44, W=44) volume; 10x10 spatial
  windows (padded to 50x50 -> 5x5 grid), each window = t*10*10 = 400 tokens of
  12-head attention (head_dim 32), followed by an output projection.

Sharding: 50 windows = 32 full (400 real tokens) + 16 edge (160) + 2 corner
(64). Each of the 8 NeuronCores processes 4 full windows + up to 3
edge/corner windows (compacted + padded to 160 tokens; the softmax
denominator is corrected by +240 to match the reference's 400-slot windows,
whose zero padding tokens each contribute exp(0)=1).

v2 pipeline (~243-270us vs the v1 baseline at ~346us):
  - All matmul operands are bf16 (x, qkv/proj weights, q, k, v, softmax
    weights, attention output). This is the single biggest win: bf16 halves
    the SBUF stream bytes per PE column and enables fast weight loads (FWL),
    where fp32r ran MMs at ~500-620ns and weight loads at ~2.5x bf16 cost.
    PSUM accumulation stays fp32; the final output is stored fp32.
  - The PE instruction stream is kept dense: the QKV projections of window
    w+1 and the normalize/project tail of window w-1 are interleaved between
    the per-head QK/exp/PV steps of window w, so the Tensor engine rarely
    starves while the Scalar engine drains softmax exps (PE idle gaps >~1us
    drop the HAM clock gate to K=4/8 = 1.2 GHz for tens of us).
  - PV is column-tiled: heads 2p/2p+1 run concurrently in array column
    strips (tile_position (0,0)/(0,64)), halving PV wall time. The 33rd
    stationary column is the all-ones softmax-denominator row, which rides
    the PV pass for free.
  - exp for 2 of 12 heads is a Schraudolph bit-trick on the Vector engine
    (es.bitcast(int16) = int16(a*scale*x + b)) to unload the Scalar engine;
    scores span only [-1.6, 1.6] and total output error stays ~1.0e-2 of
    output scale (tolerance 2e-2). GpSimd cannot read PSUM, so it only runs
    the SBUF-side rescale muls and gather/broadcast DMAs.
  - Normalization uses reciprocal_approx_fast and splits the 12 per-head
    rescale muls between Vector and GpSimd.
"""
import os
import sys

for _p in ("/opt/trn_rl_repo",):
    if os.path.isdir(_p) and _p not in sys.path:
        sys.path.append(_p)

import numpy as np
import ml_dtypes

import concourse.bass as bass
import concourse.bacc as bacc
import concourse.mybir as mybir
import concourse.tile as tile

F32 = mybir.dt.float32
F32R = mybir.dt.float32r
BF16 = mybir.dt.bfloat16
I16 = mybir.dt.int16
AF = mybir.ActivationFunctionType
ALU = mybir.AluOpType

C = 384
NH = 12
HD = 32
SCALE = HD ** -0.5
NF = 4      # full windows per core (n=400)
NS = 3      # small windows per core (padded to n=160)
NFull = 400
NSmall = 160
VW = 33     # V columns per head (32 dims + ones column for the denominator)

# Schraudolph exp constants, bf16 flavor:
# exp(x) ~= bitcast_bf16(int16(x*EXPA16 + EXPB16))
EXPA16 = 184.6649652337873     # 2^7 / ln 2
EXPB16 = 127.0 * 128.0 - 5.585


def ceil_div(a, b):
    return (a + b - 1) // b


def build_kernel(dve_exp_heads=(5, 11), pool_exp_heads=(), pv_col_tile=True):
    nc = bacc.Bacc("TRN2", target_bir_lowering=False, debug=False, num_devices=8)

    xf = nc.declare_dram_parameter("xf", [NF, 128, 3, NFull], BF16, isOutput=False)
    xs = nc.declare_dram_parameter("xs", [NS, 128, 3, NSmall], BF16, isOutput=False)
    wq = nc.declare_dram_parameter("wq", [128, 3, C], BF16, isOutput=False)
    wk = nc.declare_dram_parameter("wk", [128, 3, C], BF16, isOutput=False)
    wv = nc.declare_dram_parameter("wv", [128, 3, C], BF16, isOutput=False)
    wp = nc.declare_dram_parameter("wp", [128, 3, C], BF16, isOutput=False)
    pb = nc.declare_dram_parameter("pb", [128, 3], F32, isOutput=False)
    zf = nc.declare_dram_parameter("zf", [NF, 128, 3, NFull], F32, isOutput=True)
    zs = nc.declare_dram_parameter("zs", [NS, 128, 3, NSmall], F32, isOutput=True)

    slots = [(s, NFull, xf, zf, s) for s in range(NF)] + \
            [(NF + s, NSmall, xs, zs, s) for s in range(NS)]
    NW = len(slots)

    with tile.TileContext(nc) as tc:
        with tc.tile_pool(name="weights", bufs=1) as wpool, \
             tc.tile_pool(name="xio", bufs=3) as xpool, \
             tc.tile_pool(name="qk", bufs=2) as qkpool, \
             tc.tile_pool(name="vaug", bufs=2) as vpool, \
             tc.tile_pool(name="es", bufs=4) as espool, \
             tc.tile_pool(name="oun", bufs=2) as ounpool, \
             tc.tile_pool(name="oz", bufs=2) as ozpool, \
             tc.tile_pool(name="nrm", bufs=2) as nrmpool, \
             tc.tile_pool(name="nrmbig", bufs=2) as nbpool, \
             tc.tile_pool(name="dscratch", bufs=2, space="DRAM") as dpool, \
             tc.tile_pool(name="ps_s", bufs=2, space="PSUM") as ps_s, \
             tc.tile_pool(name="ps_pv", bufs=2, space="PSUM") as ps_pv, \
             tc.tile_pool(name="ps_mm", bufs=2, space="PSUM") as ps_mm:

            twq = wpool.tile([128, 3, C], BF16, tag="wq")
            twk = wpool.tile([128, 3, C], BF16, tag="wk")
            twv = wpool.tile([128, 3, C], BF16, tag="wv")
            twp = wpool.tile([128, 3, C], BF16, tag="wp")
            tpb = wpool.tile([128, 3], F32, tag="pb")
            # garbage operand for PE keep-warm filler matmuls (see below)
            twarm = wpool.tile([128, 512], BF16, tag="warm")
            nc.vector.memset(twarm[:], 0.0)

            class Window:
                def __init__(self, w):
                    self.w = w
                    self.slot, self.n, self.xin, self.zout, self.si = slots[w]
                    self.n_mt = ceil_div(self.n, 128)
                    self.m_sizes = [min(128, self.n - 128 * j)
                                    for j in range(self.n_mt)]
                    self.full = self.n == NFull
                    self.es = {}

                # ---- stage 1: x load + QKV projections (run during w-1) ----
                def load_x(self):
                    self.xt = xpool.tile([128, 3, NFull], BF16, tag="xt",
                                         name=f"xt{self.w}")
                    nc.sync.dma_start(out=self.xt[:, :, 0:self.n],
                                      in_=self.xin[self.si])

                def qkv_chunks(self):
                    n = self.n
                    out = []

                    def qk_proj(dst_key, i, self=self):
                        if dst_key not in ("qt", "kt"):
                            raise ValueError
                        if not hasattr(self, dst_key):
                            setattr(self, dst_key,
                                    qkpool.tile([128, 3, NFull], BF16,
                                                tag=dst_key,
                                                name=f"{dst_key}{self.w}"))
                        dst = getattr(self, dst_key)
                        w_t = twq if dst_key == "qt" else twk
                        pmm = ps_mm.tile([128, 512], F32, tag="mm")
                        for kk in range(3):
                            nc.tensor.matmul(pmm[:, 0:n],
                                             w_t[:, kk, 128 * i:128 * i + 128],
                                             self.xt[:, kk, 0:n],
                                             start=(kk == 0), stop=(kk == 2))
                        nc.vector.tensor_copy(dst[:, i, 0:n], pmm[:, 0:n])

                    def v_tile(j, self=self):
                        if not hasattr(self, "vg"):
                            self.vg = vpool.tile([128, 4, NH * VW], BF16,
                                                 tag="vg", name=f"vg{self.w}")
                        mj = self.m_sizes[j]
                        pmm = ps_mm.tile([128, 512], F32, tag="mm")
                        for kk in range(3):
                            nc.tensor.matmul(pmm[0:mj, 0:C],
                                             self.xt[:, kk, 128 * j:128 * j + mj],
                                             twv[:, kk, :],
                                             start=(kk == 0), stop=(kk == 2))
                        vslice = self.vg[0:mj, j, :].rearrange(
                            "p (h c) -> p h c", h=NH)
                        nc.vector.tensor_copy(
                            vslice[:, :, 0:32],
                            pmm[0:mj, 0:C].rearrange("p (h c) -> p h c", h=NH))
                        nc.vector.memset(vslice[:, :, 32:33], 1.0)

                    for dst in ("qt", "kt"):
                        for i in range(3):
                            out.append(lambda d=dst, i=i: qk_proj(d, i))
                    for j in range(self.n_mt):
                        out.append(lambda j=j: v_tile(j))
                    return out

                # ---- stage 2: per-head QK^T + exp ----
                def head(self, h):
                    n, n_mt = self.n, self.n_mt
                    ti, to = h // 4, 32 * (h % 4)
                    es = espool.tile([128, 4, NFull], BF16, tag="es",
                                     name=f"es{self.w}_{h}")
                    self.es[h] = es
                    exp_eng = None
                    if self.full:
                        if h in dve_exp_heads:
                            exp_eng = nc.vector
                        elif h in pool_exp_heads:
                            exp_eng = nc.gpsimd
                    for half in range(ceil_div(n_mt, 2)):
                        pss = ps_s.tile([128, 2, 512], F32, tag="s")
                        jj = [j for j in (2 * half, 2 * half + 1) if j < n_mt]
                        for idx, j in enumerate(jj):
                            mj = self.m_sizes[j]
                            nc.tensor.matmul(
                                pss[0:mj, idx, 0:n],
                                self.kt[to:to + 32, ti, 128 * j:128 * j + mj],
                                self.qt[to:to + 32, ti, 0:n],
                                start=True, stop=True, tile_position=(to, 0))
                        dst = es[:, 2 * half:2 * half + len(jj), 0:n]
                        src = pss[:, 0:len(jj), 0:n]
                        if exp_eng is None:
                            nc.scalar.activation(dst, src, AF.Exp, scale=SCALE)
                        else:
                            exp_eng.tensor_scalar(
                                dst.bitcast(I16), src,
                                SCALE * EXPA16, EXPB16, ALU.mult, ALU.add)

                # ---- stage 3: PV for a head pair, column-tiled ----
                def pv_pair(self, p):
                    n, n_mt = self.n, self.n_mt
                    h0, h1 = 2 * p, 2 * p + 1
                    if not hasattr(self, "oun"):
                        self.oun = ounpool.tile([128, 6, NFull], F32R,
                                                tag="oun", name=f"oun{self.w}")
                    es0, es1 = self.es.pop(h0), self.es.pop(h1)
                    if pv_col_tile:
                        ppv = ps_pv.tile([128, 512], F32, tag="pv")
                        for j in range(n_mt):
                            mj = self.m_sizes[j]
                            nc.tensor.matmul(
                                ppv[0:33, 0:n],
                                self.vg[0:mj, j, VW * h0:VW * h0 + VW],
                                es0[0:mj, j, 0:n],
                                start=(j == 0), stop=(j == n_mt - 1),
                                tile_position=(0, 0), skip_group_check=True)
                            nc.tensor.matmul(
                                ppv[64:97, 0:n],
                                self.vg[0:mj, j, VW * h1:VW * h1 + VW],
                                es1[0:mj, j, 0:n],
                                start=(j == 0), stop=(j == n_mt - 1),
                                tile_position=(0, 64), skip_group_check=True)
                        nc.vector.tensor_copy(self.oun[0:33, p, 0:n],
                                              ppv[0:33, 0:n])
                        nc.vector.tensor_copy(self.oun[64:97, p, 0:n],
                                              ppv[64:97, 0:n])
                    else:
                        for a, (hh, ee) in enumerate(((h0, es0), (h1, es1))):
                            ppv = ps_pv.tile([128, 512], F32, tag="pv")
                            for j in range(n_mt):
                                mj = self.m_sizes[j]
                                nc.tensor.matmul(
                                    ppv[0:33, 0:n],
                                    self.vg[0:mj, j, VW * hh:VW * hh + VW],
                                    ee[0:mj, j, 0:n],
                                    start=(j == 0), stop=(j == n_mt - 1))
                            nc.vector.tensor_copy(
                                self.oun[64 * a:64 * a + 33, p, 0:n],
                                ppv[0:33, 0:n])

                # ---- stage 4: normalize + project + store (run during w+1) --
                def t_dal(self):
                    n = self.n
                    self.dal = nrmpool.tile([12, NFull], F32, tag="dal",
                                            name=f"dal{self.w}")
                    nc.gpsimd.dma_start(out=self.dal[0:6, 0:n],
                                        in_=self.oun[32:33, :, 0:n])
                    nc.gpsimd.dma_start(out=self.dal[6:12, 0:n],
                                        in_=self.oun[96:97, :, 0:n])

                def t_rcp(self):
                    n = self.n
                    rcp = nrmpool.tile([12, NFull], F32, tag="rcp",
                                       name=f"rcp{self.w}")
                    self.rcp = rcp
                    src = self.dal
                    if not self.full:
                        dfl = nrmpool.tile([12, NFull], F32, tag="dfl",
                                           name=f"dfl{self.w}")
                        nc.vector.tensor_scalar_add(
                            dfl[:, 0:n], self.dal[:, 0:n],
                            float(NFull - NSmall))
                        src = dfl
                    nc.vector.reciprocal_approx_fast(rcp[:, 0:n], src[:, 0:n])

                def _bca_tile(self):
                    if not hasattr(self, "bca"):
                        # bca[64a+b, p, :] = 1/den of head 2p+a, so the mul
                        # input bases match oun's (same-start-partition rule)
                        self.bca = nbpool.tile([128, 6, NFull], F32, tag="bca",
                                               name=f"bca{self.w}")

                def t_bcast(self):
                    n = self.n
                    dsc = dpool.tile([12, NFull], F32, tag="dsc",
                                     name=f"dsc{self.w}")
                    nc.gpsimd.dma_start(out=dsc[:, 0:n], in_=self.rcp[:, 0:n])
                    self._bca_tile()
                    for a in range(2):
                        nc.gpsimd.dma_start(
                            out=self.bca[64 * a:64 * a + 32, :, 0:n],
                            in_=dsc[None, 6 * a:6 * a + 6, 0:n]
                            .to_broadcast((32, 6, n)))

                def tail_half(self, half):
                    # half-batched dal->rcp->broadcast chain for pairs
                    # 3*half..3*half+2, used for the last window so its tail
                    # pipelines into the head loop instead of serializing
                    # after it (5 DMA triggers per half)
                    n = self.n
                    p0 = 3 * half
                    dal_h = nrmpool.tile([6, NFull], F32, tag="dalh",
                                         name=f"dalh{self.w}_{half}")
                    nc.gpsimd.dma_start(out=dal_h[0:3, 0:n],
                                        in_=self.oun[32:33, p0:p0 + 3, 0:n])
                    nc.gpsimd.dma_start(out=dal_h[3:6, 0:n],
                                        in_=self.oun[96:97, p0:p0 + 3, 0:n])
                    src = dal_h
                    if not self.full:
                        dfl_h = nrmpool.tile([6, NFull], F32, tag="dflh",
                                             name=f"dflh{self.w}_{half}")
                        nc.vector.tensor_scalar_add(
                            dfl_h[:, 0:n], dal_h[:, 0:n],
                            float(NFull - NSmall))
                        src = dfl_h
                    rcp_h = nrmpool.tile([6, NFull], F32, tag="rcph",
                                         name=f"rcph{self.w}_{half}")
                    nc.vector.reciprocal_approx_fast(rcp_h[:, 0:n],
                                                     src[:, 0:n])
                    dsc_h = dpool.tile([6, NFull], F32, tag="dsch",
                                       name=f"dsch{self.w}_{half}")
                    nc.gpsimd.dma_start(out=dsc_h[:, 0:n], in_=rcp_h[:, 0:n])
                    self._bca_tile()
                    for a in range(2):
                        nc.gpsimd.dma_start(
                            out=self.bca[64 * a:64 * a + 32, p0:p0 + 3, 0:n],
                            in_=dsc_h[None, 3 * a:3 * a + 3, 0:n]
                            .to_broadcast((32, 3, n)))

                def t_mul(self, h):
                    n = self.n
                    ti, to = h // 4, 32 * (h % 4)
                    p, a = h // 2, h % 2
                    if not hasattr(self, "ot"):
                        self.ot = ozpool.tile([128, 3, NFull], BF16,
                                              tag="ot", name=f"ot{self.w}")
                    eng = nc.vector if h % 2 == 0 else nc.gpsimd
                    eng.tensor_mul(
                        self.ot[to:to + 32, ti, 0:n],
                        self.oun[64 * a:64 * a + 32, p, 0:n],
                        self.bca[64 * a:64 * a + 32, p, 0:n].bitcast(F32R))

                def t_proj(self, i):
                    n = self.n
                    if not hasattr(self, "zt"):
                        self.zt = ozpool.tile([128, 3, NFull], F32,
                                              tag="zt", name=f"zt{self.w}")
                    pmm = ps_mm.tile([128, 512], F32, tag="mm")
                    for kk in range(3):
                        nc.tensor.matmul(pmm[:, 0:n],
                                         twp[:, kk, 128 * i:128 * i + 128],
                                         self.ot[:, kk, 0:n],
                                         start=(kk == 0), stop=(kk == 2))
                    nc.vector.tensor_scalar_add(self.zt[:, i, 0:n],
                                                pmm[:, 0:n],
                                                tpb[:, i:i + 1])

                def t_store(self):
                    nc.sync.dma_start(out=self.zout[self.si],
                                      in_=self.zt[:, :, 0:self.n])

                def tail_chunks(self):
                    out = [self.t_dal, self.t_rcp, self.t_bcast]
                    for h in range(NH):
                        out.append(lambda h=h: self.t_mul(h))
                    for i in range(3):
                        out.append(lambda i=i: self.t_proj(i))
                    out.append(self.t_store)
                    return out

            wins = [Window(w) for w in range(NW)]

            # ---- prologue: x0 + first weights on the sync queue, the
            # rest on the gpsimd queue so transfers overlap ----
            wins[0].load_x()
            for t, src in ((twq, wq), (twk, wk), (twv, wv)):
                nc.sync.dma_start(out=t[:], in_=src[:])
            for t, src in ((twp, wp), (tpb, pb)):
                nc.gpsimd.dma_start(out=t[:], in_=src[:])
            for c in wins[0].qkv_chunks():
                c()

            for w in range(NW):
                cur = wins[w]
                filler = []
                reserved = []
                if w + 1 < NW:
                    nxt = wins[w + 1]
                    filler.append(nxt.load_x)
                    filler.extend(nxt.qkv_chunks())
                if w > 0:
                    filler.extend(wins[w - 1].tail_chunks())
                # Distribute filler chunks across the 12 head slots. The
                # chains (x-load before qkv; dal->rcp->bcast->muls->proj)
                # are kept in order; we round-robin merge the two lists so
                # both PE filler (qkv/proj matmuls) and the normalize tail
                # spread evenly across the window.
                nslots = NH
                per_slot = [[] for _ in range(nslots)]
                for idx, c in enumerate(filler):
                    per_slot[min(nslots - 1, idx * nslots // max(1, len(filler)))].append(c)
                last = w == NW - 1
                for h in range(NH):
                    cur.head(h)
                    if not cur.full:
                        # small windows have too little real PE work per head
                        # to keep the HAM clock gate at K=8/8; pad the stream
                        # with dependency-free matmuls that fill would-be
                        # idle slots (no consumers, no cross-engine waits)
                        wps = ps_mm.tile([128, 512], F32, tag="mm")
                        for r in range(2):
                            nc.tensor.matmul(wps[:, 0:512], twarm[:, 0:128],
                                             twarm[:, 0:512],
                                             start=True, stop=True)
                    if h >= 2 and h % 2 == 0:
                        p = (h - 2) // 2
                        cur.pv_pair(p)
                        if last and p == 2:
                            cur.tail_half(0)
                            for hh in range(6):
                                cur.t_mul(hh)
                    for c in per_slot[h]:
                        c()
                cur.pv_pair(5)
                if last:
                    cur.tail_half(1)
                    for hh in range(6, NH):
                        cur.t_mul(hh)
                    for i in range(3):
                        cur.t_proj(i)
                    cur.t_store()

    nc.compile()
    return nc




WS = 10
NH = 12
C = 384
B, T, H, W = 2, 4, 44, 44
HG = WG = 5
NFull = 400
NSmall = 160
NF, NS = 4, 3


def window_partition(x):
    """x: (B*T, H*W, C) -> windows (B, 25, 400, C) padded, plus metadata."""
    ax = x.reshape(B, T, H, W, C)
    pad = WS * HG
    axp = np.zeros((B, T, pad, pad, C), dtype=x.dtype)
    axp[:, :, :H, :W, :] = ax
    axp = axp.reshape(B, T, HG, WS, WG, WS, C)
    axp = axp.transpose(0, 2, 4, 1, 3, 5, 6).reshape(B, HG * WG, T * WS * WS, C)
    return axp


def classify_windows():
    """Return (full_list, small_list) of (b, w, n_valid)."""
    full, small = [], []
    for b in range(B):
        for i in range(HG):
            for j in range(WG):
                w = i * WG + j
                vi = min(WS, H - i * WS)
                vj = min(WS, W - j * WS)
                nv = T * vi * vj
                if vi == WS and vj == WS:
                    full.append((b, w))
                else:
                    small.append((b, w, nv))
    return full, small


def window_token_index(w):
    """For window w, indices of its 400 token slots ordered by (t, wi, wj),
    and validity mask."""
    i, j = w // WG, w % WG
    idx = np.zeros((T, WS, WS), dtype=np.int64)
    valid = np.zeros((T, WS, WS), dtype=bool)
    for t in range(T):
        for a in range(WS):
            for bb in range(WS):
                hh, ww = i * WS + a, j * WS + bb
                ok = (hh < H) and (ww < W)
                valid[t, a, bb] = ok
                idx[t, a, bb] = (t * H + min(hh, H - 1)) * W + min(ww, W - 1)
    return idx.reshape(-1), valid.reshape(-1)


def compact_window_tokens(xw, w):
    """xw: (400, C) padded window tokens (zeros at invalid). Returns
    (n_valid tokens compacted, order) where order lists the valid slot ids."""
    _, valid = window_token_index(w)
    order = np.nonzero(valid)[0]
    return xw[order], order


def shard_inputs(x, qkv_w, proj_w, proj_b):
    """Build per-core in_maps. Returns (in_maps, meta) where meta is used by
    unshard."""
    x = np.asarray(x, dtype=np.float32)
    xw = window_partition(x)           # (B, 25, 400, C)
    full, small = classify_windows()
    assert len(full) == 32 and len(small) == 18

    # per-core assignment: 4 full, and up to 3 small (pad with zero windows)
    full_assign = [full[4 * c:4 * c + 4] for c in range(8)]
    small_assign = [[] for _ in range(8)]
    for k, s in enumerate(small):
        small_assign[k % 8].append(s)
    meta = {"full": full_assign, "small": small_assign, "orders": {}}

    wqT = qkv_w[0:C, :].T.astype(np.float32)      # (C, C): [c, qf]
    wkT = qkv_w[C:2 * C, :].T.astype(np.float32)
    wvT = qkv_w[2 * C:3 * C, :].T.astype(np.float32)
    wpT = proj_w.T.astype(np.float32)

    def wtile(wt):  # (C=384 rows c, C cols f) -> [128, 3, 384]
        return np.ascontiguousarray(
            wt.reshape(3, 128, C).transpose(1, 0, 2)).astype(ml_dtypes.bfloat16)

    in_maps = []
    for c in range(8):
        xf = np.zeros((NF, 128, 3, NFull), dtype=ml_dtypes.bfloat16)
        for s, (b, w) in enumerate(full_assign[c]):
            xt = xw[b, w].T                      # (C, 400)
            xf[s] = xt.reshape(3, 128, NFull).transpose(1, 0, 2).astype(ml_dtypes.bfloat16)
        xs = np.zeros((NS, 128, 3, NSmall), dtype=ml_dtypes.bfloat16)
        for s, (b, w, nv) in enumerate(small_assign[c]):
            toks, order = compact_window_tokens(xw[b, w], w)
            meta["orders"][(b, w)] = order
            xt = np.zeros((C, NSmall), dtype=np.float32)
            xt[:, 0:nv] = toks.T
            xs[s] = xt.reshape(3, 128, NSmall).transpose(1, 0, 2).astype(ml_dtypes.bfloat16)
        in_maps.append({
            "xf": xf, "xs": xs,
            "wq": wtile(wqT), "wk": wtile(wkT), "wv": wtile(wvT),
            "wp": wtile(wpT),
            "pb": np.ascontiguousarray(proj_b.astype(np.float32).reshape(3, 128).T),
        })
    return in_maps, meta


def unshard_outputs(results, meta):
    """results: list of 8 dicts with zf (NF,128,3,400), zs. Return (B*T, H*W, C)."""
    zwin = np.zeros((B, HG * WG, T * WS * WS, C), dtype=np.float32)
    for c in range(8):
        zfc, zsc = results[c]["zf"], results[c]["zs"]
        for s, (b, w) in enumerate(meta["full"][c]):
            zt = zfc[s].transpose(1, 0, 2).reshape(C, NFull)   # (C, 400)
            zwin[b, w] = zt.T
        for s, (b, w, nv) in enumerate(meta["small"][c]):
            zt = zsc[s].transpose(1, 0, 2).reshape(C, NSmall)
            order = meta["orders"][(b, w)]
            zwin[b, w][order] = zt.T[0:nv]
    # reverse window partition
    z = zwin.reshape(B, HG, WG, T, WS, WS, C)
    z = z.transpose(0, 3, 1, 4, 2, 5, 6).reshape(B, T, HG * WS, WG * WS, C)
    z = z[:, :, :H, :W, :]
    return z.reshape(B * T, H * W, C)


_CACHE = {}


def _get_nc():
    if "nc" not in _CACHE:
        _CACHE["nc"] = build_kernel()
    return _CACHE["nc"]


def kernel(x, qkv_w, proj_w, proj_b, t=4, H=44, W=44, **_unused):
    from concourse.bass_utils import run_bass_kernel_spmd

    x = np.asarray(x, dtype=np.float32)
    qkv_w = np.asarray(qkv_w, dtype=np.float32)
    proj_w = np.asarray(proj_w, dtype=np.float32)
    proj_b = np.asarray(proj_b, dtype=np.float32)
    in_maps, meta = shard_inputs(x, qkv_w, proj_w, proj_b)
    nc = _get_nc()
    res = run_bass_kernel_spmd(nc, in_maps, list(range(8)))
    return unshard_outputs(res.results, meta)
